# revision 5
# baseline (speedup 1.0000x reference)
"""Trainium2 Bass kernel for the all-pairs cosine-similarity loss.

Reference computes:  loss = mean_{i<j}(1 - cos(f_i, f_j))
Closed form used here (mathematically identical for nonzero rows):
    u_i = f_i / ||f_i||           (normalized rows)
    g   = sum_i u_i               (D-vector)
    sum_{i<j} cos(f_i,f_j) = (||g||^2 - N) / 2
    loss = 1 - (||g||^2 - N) / (2 * num_pairs)

This turns an O(N^2 D) matmul problem into an O(N D) memory-bound pass:
each core streams its 512-row shard once, computes row norms, does a
weighted row-sum on the tensor engine (w = 1/||f_i|| as the stationary
operand), AllGathers the 8 partial [1024] vectors, and finishes the
scalar on-device.
"""

import numpy as np

N = 4096
D = 1024
N_CORES = 8
ROWS = N // N_CORES          # 512 rows per core
P = 128                      # SBUF partitions
T = ROWS // P                # 4 row-tiles of [128, D] per core
NUM_PAIRS = N * (N - 1) // 2

_LOSS_SCALE = -1.0 / (2.0 * NUM_PAIRS)
_LOSS_BIAS = 1.0 + N / (2.0 * NUM_PAIRS)

_built = None


def _build(collective: bool = True):
    import concourse.bacc as bacc
    import concourse.mybir as mybir
    import concourse.tile as tile

    f32 = mybir.dt.float32
    nc = bacc.Bacc(
        "TRN2", target_bir_lowering=False, debug=False, num_devices=N_CORES
    )

    feats = nc.dram_tensor("feats", [ROWS, D], f32, kind="ExternalInput")
    loss_out = nc.dram_tensor("loss", [1, 1], f32, kind="ExternalOutput")
    # Internal DRAM bounce buffers for the collective (I/O tensors are not
    # legal collective operands; output must be in the Shared scratchpad).
    g_local = nc.dram_tensor("g_local", [1, D], f32)
    g_all = nc.dram_tensor("g_all", [N_CORES, D], f32, addr_space="Shared")

    with tile.TileContext(nc) as tc:
        with (
            tc.tile_pool(name="pool", bufs=1) as pool,
            tc.tile_pool(name="psum", bufs=1, space="PSUM") as psum,
        ):
            fview = feats.ap().rearrange("(t p) d -> t p d", p=P)
            ftiles = []
            for t in range(T):
                ft = pool.tile([P, D], f32, tag=f"f{t}", name=f"ft{t}")
                nc.sync.dma_start(ft[:], fview[t])
                ftiles.append(ft)

            # Row sums-of-squares, one column per row-tile, on ACT
            # (Square + accum_out does square+rowsum in one pass).
            # NB: vector.tensor_tensor_reduce crashes the NRT worker on
            # this runtime — keep to ACT/standard DVE instructions.
            sq = pool.tile([P, T], f32, tag="sq")
            sc_a = pool.tile([P, D], f32, tag="sc_a")
            for t in range(T):
                nc.scalar.activation(
                    sc_a[:],
                    ftiles[t][:],
                    mybir.ActivationFunctionType.Square,
                    accum_out=sq[:, t : t + 1],
                )

            # w = 1/sqrt(sq) ;  sqrt on ACT, exact reciprocal on DVE
            nrm = pool.tile([P, T], f32, tag="nrm")
            w = pool.tile([P, T], f32, tag="w")
            nc.scalar.sqrt(nrm[:], sq[:])
            nc.vector.reciprocal(w[:], nrm[:])

            # Partial g = sum_i w_i * f_i via PE: out[1, D] = w[:,t].T @ F_t
            gp = psum.tile([1, D], f32, tag="gp")
            for t in range(T):
                for h in range(2):
                    nc.tensor.matmul(
                        gp[:, h * 512 : (h + 1) * 512],
                        w[:, t : t + 1],
                        ftiles[t][:, h * 512 : (h + 1) * 512],
                        start=(t == 0),
                        stop=(t == T - 1),
                    )

            gs = pool.tile([1, D], f32, tag="gs")
            nc.scalar.copy(gs[:], gp[:])
            nc.sync.dma_start(g_local.ap(), gs[:])

            if collective:
                nc.gpsimd.collective_compute(
                    "AllGather",
                    mybir.AluOpType.bypass,
                    replica_groups=[list(range(N_CORES))],
                    ins=[g_local.ap().opt()],
                    outs=[g_all.ap().opt()],
                )
            else:
                # timing-model variant (TimelineSim can't simulate
                # collectives): stand-in DMA with the same data deps
                nc.sync.dma_start(g_all.ap()[0:1], g_local.ap())

            # Bring the 8 partials in as [8, D], reduce ranks on PE with a
            # ones vector, then square-reduce the [1, D] result on DVE.
            ga = pool.tile([N_CORES, D], f32, tag="ga")
            nc.sync.dma_start(ga[:], g_all.ap())
            ones8 = pool.tile([N_CORES, 1], f32, tag="ones8")
            nc.gpsimd.memset(ones8[:], 1.0)

            gt = psum.tile([1, D], f32, tag="gt")
            for h in range(2):
                nc.tensor.matmul(
                    gt[:, h * 512 : (h + 1) * 512],
                    ones8[:],
                    ga[:, h * 512 : (h + 1) * 512],
                    start=True,
                    stop=True,
                )

            sc_g = pool.tile([1, D], f32, tag="sc_g")
            gg = pool.tile([1, 1], f32, tag="gg")
            nc.scalar.activation(
                sc_g[:],
                gt[:],
                mybir.ActivationFunctionType.Square,
                accum_out=gg[:],
            )

            # loss = 1 - (gg - N) / (2*num_pairs)  ==  gg*scale + bias
            loss_sb = pool.tile([1, 1], f32, tag="loss_sb")
            nc.scalar.activation(
                loss_sb[:],
                gg[:],
                mybir.ActivationFunctionType.Copy,
                bias=_LOSS_BIAS,
                scale=_LOSS_SCALE,
            )
            nc.sync.dma_start(loss_out.ap(), loss_sb[:])

    nc.compile()
    return nc


def _get_nc():
    global _built
    if _built is None:
        _built = _build()
    return _built


def kernel(feats: np.ndarray) -> np.ndarray:
    from concourse import bass_utils

    nc = _get_nc()
    feats = np.ascontiguousarray(np.asarray(feats, dtype=np.float32))
    assert feats.shape == (N, D), feats.shape

    in_maps = [
        {"feats": feats[c * ROWS : (c + 1) * ROWS]} for c in range(N_CORES)
    ]
    res = bass_utils.run_bass_kernel_spmd(
        nc, in_maps, core_ids=list(range(N_CORES))
    )
    out = res.results[0]["loss"]
    return np.float32(out.reshape(())[()])


# revision 8
# speedup vs baseline: 1.9419x; 1.9419x over previous
"""Trainium2 Bass kernel for the all-pairs cosine-similarity loss.

Reference computes:  loss = mean_{i<j}(1 - cos(f_i, f_j))
Closed form used here (mathematically identical for nonzero rows):
    u_i = f_i / ||f_i||           (normalized rows)
    g   = sum_i u_i               (D-vector)
    sum_{i<j} cos(f_i,f_j) = (||g||^2 - N) / 2
    loss = 1 - (||g||^2 - N) / (2 * num_pairs)

This turns an O(N^2 D) matmul problem into an O(N D) memory-bound pass:
each core streams its 512-row shard once (cast to bf16 during the DMA),
computes row norms (ACT square+accum), does a weighted row-sum on the
tensor engine (w = 1/||f_i|| stationary, bf16), AllGathers the 8
partial [1024] vectors, and finishes the scalar on-device.

bf16 note: the matmul operands are bf16 but every accumulation is fp32
(PSUM / accum_out).  The loss is 1 + O(1e-5) and the bf16 rounding of
unit-normalized rows perturbs it by ~1e-7 — far below the fp32
rounding noise of the reference's own 16M-element reduction.
"""

import numpy as np

N = 4096
D = 1024
N_CORES = 8
ROWS = N // N_CORES          # 512 rows per core
P = 128                      # SBUF partitions
T = ROWS // P                # 4 row-tiles of [128, D] per core
NUM_PAIRS = N * (N - 1) // 2

_LOSS_SCALE = -1.0 / (2.0 * NUM_PAIRS)
_LOSS_BIAS = 1.0 + N / (2.0 * NUM_PAIRS)

_built = None


def _build(collective: bool = True):
    import concourse.bacc as bacc
    import concourse.mybir as mybir
    import concourse.tile as tile

    f32 = mybir.dt.float32
    bf16 = mybir.dt.bfloat16
    nc = bacc.Bacc(
        "TRN2", target_bir_lowering=False, debug=False, num_devices=N_CORES
    )

    feats = nc.dram_tensor("feats", [ROWS, D], f32, kind="ExternalInput")
    loss_out = nc.dram_tensor("loss", [1, 1], f32, kind="ExternalOutput")
    # Internal DRAM bounce buffers for the collective (I/O tensors are not
    # legal collective operands; output must be in the Shared scratchpad).
    g_local = nc.dram_tensor("g_local", [1, D], f32)
    g_all = nc.dram_tensor("g_all", [N_CORES, D], f32, addr_space="Shared")

    with tile.TileContext(nc) as tc:
        with (
            tc.tile_pool(name="pool", bufs=1) as pool,
            tc.tile_pool(name="psum", bufs=1, space="PSUM") as psum,
        ):
            # Warm both ACT function-table sets (Square / Sqrt+Copy) while
            # the input DMAs stream — otherwise the 1.3us table load for
            # Sqrt lands on the critical path between squares and matmuls.
            dummy = pool.tile([1, 1], f32, tag="dummy")
            nc.gpsimd.memset(dummy[:], 1.0)
            nc.scalar.square(dummy[:], dummy[:])
            nc.scalar.sqrt(dummy[:], dummy[:])

            # Load + cast f32 -> bf16 during the DMA (SWDGE handles the
            # dtype conversion inline).
            fview = feats.ap().rearrange("(t p) d -> t p d", p=P)
            ftiles = []
            for t in range(T):
                ft = pool.tile([P, D], bf16, tag=f"f{t}", name=f"ft{t}")
                nc.gpsimd.dma_start(ft[:], fview[t])
                ftiles.append(ft)

            # Per-tile chains: square+rowsum (ACT, fp32 accum) -> sqrt (ACT)
            # -> reciprocal (DVE) -> bf16 cast (DVE) -> PE matmul pair.
            # Per-tile (not batched) so tile t's matmuls start as soon as
            # its own norm is ready instead of waiting on all 4 squares.
            # NB: vector.tensor_tensor_reduce crashes the NRT worker on
            # this runtime — keep to ACT/standard DVE instructions.
            sq = pool.tile([P, T], f32, tag="sq")
            nrm = pool.tile([P, T], f32, tag="nrm")
            w = pool.tile([P, T], f32, tag="w")
            wb = pool.tile([P, T], bf16, tag="wb")
            sc_a = pool.tile([P, D], bf16, tag="sc_a")
            gp = psum.tile([1, D], f32, tag="gp")
            for t in range(T):
                ts = slice(t, t + 1)
                nc.scalar.activation(
                    sc_a[:],
                    ftiles[t][:],
                    mybir.ActivationFunctionType.Square,
                    accum_out=sq[:, ts],
                )
                nc.scalar.sqrt(nrm[:, ts], sq[:, ts])
                nc.vector.reciprocal(w[:, ts], nrm[:, ts])
                nc.vector.tensor_copy(wb[:, ts], w[:, ts])
                for h in range(2):
                    nc.tensor.matmul(
                        gp[:, h * 512 : (h + 1) * 512],
                        wb[:, ts],
                        ftiles[t][:, h * 512 : (h + 1) * 512],
                        start=(t == 0),
                        stop=(t == T - 1),
                    )

            # PSUM -> SBUF (split across ACT+DVE) -> DRAM (dma_start can't
            # source PSUM).
            gs = pool.tile([1, D], f32, tag="gs")
            nc.scalar.copy(gs[:, 0:512], gp[:, 0:512])
            nc.vector.tensor_copy(gs[:, 512:D], gp[:, 512:D])
            nc.sync.dma_start(g_local.ap(), gs[:])

            if collective:
                nc.gpsimd.collective_compute(
                    "AllGather",
                    mybir.AluOpType.bypass,
                    replica_groups=[list(range(N_CORES))],
                    ins=[g_local.ap().opt()],
                    outs=[g_all.ap().opt()],
                )
            else:
                # timing-model variant (TimelineSim can't simulate
                # collectives): stand-in DMA with the same data deps
                nc.sync.dma_start(g_all.ap()[0:1], g_local.ap())

            # Bring the 8 partials in as [8, D] bf16 (cast on load),
            # reduce ranks on PE with a ones vector, square-reduce on ACT.
            ga = pool.tile([N_CORES, D], bf16, tag="ga")
            nc.gpsimd.dma_start(ga[:], g_all.ap())
            ones8 = pool.tile([N_CORES, 1], bf16, tag="ones8")
            nc.gpsimd.memset(ones8[:], 1.0)

            gt = psum.tile([1, D], f32, tag="gt")
            for h in range(2):
                nc.tensor.matmul(
                    gt[:, h * 512 : (h + 1) * 512],
                    ones8[:],
                    ga[:, h * 512 : (h + 1) * 512],
                    start=True,
                    stop=True,
                )

            sc_g = pool.tile([1, D], f32, tag="sc_g")
            gg = pool.tile([1, 1], f32, tag="gg")
            nc.scalar.activation(
                sc_g[:],
                gt[:],
                mybir.ActivationFunctionType.Square,
                accum_out=gg[:],
            )

            # loss = 1 - (gg - N) / (2*num_pairs)  ==  gg*scale + bias
            loss_sb = pool.tile([1, 1], f32, tag="loss_sb")
            nc.scalar.activation(
                loss_sb[:],
                gg[:],
                mybir.ActivationFunctionType.Copy,
                bias=_LOSS_BIAS,
                scale=_LOSS_SCALE,
            )
            nc.sync.dma_start(loss_out.ap(), loss_sb[:])

    nc.compile()
    return nc


def _get_nc():
    global _built
    if _built is None:
        _built = _build()
    return _built


def kernel(feats: np.ndarray) -> np.ndarray:
    from concourse import bass_utils

    nc = _get_nc()
    feats = np.ascontiguousarray(np.asarray(feats, dtype=np.float32))
    assert feats.shape == (N, D), feats.shape

    in_maps = [
        {"feats": feats[c * ROWS : (c + 1) * ROWS]} for c in range(N_CORES)
    ]
    res = bass_utils.run_bass_kernel_spmd(
        nc, in_maps, core_ids=list(range(N_CORES))
    )
    out = res.results[0]["loss"]
    return np.float32(out.reshape(())[()])
